# revision 1
# baseline (speedup 1.0000x reference)
"""KREmbedding kernel for Trainium2 (8 NeuronCores, data-parallel over batch).

reference math (f32):
    ctx = W[context]            # [B, C, D]
    cen = W[center]             # [B, D]
    dsq = sum((ctx-cen)^2, -1)  # [B, C]
    w = exp(-dsq/2); w /= (sum(w, -1) + 1e-8)
    out = sum(w[...,None]*ctx, -2)   # [B, D]

Device layout per core (B_core=1024): 8 groups x 128 batches (partition=batch).
Per group: 32 indirect row-gathers of W (one per context slot, 128 rows each)
+ 1 center gather; DVE subtract, ACT square+accumulate -> dsq; ACT exp;
DVE reduce + reciprocal for the normalizer; ACT per-partition-scalar multiply
+ DVE add for the weighted sum.
"""
import sys

for _p in ("/opt/trn_rl_repo",):
    if _p not in sys.path:
        sys.path.insert(0, _p)

import numpy as np
from contextlib import ExitStack

import concourse.bass as bass
import concourse.tile as tile
from concourse import bacc, mybir

V, D = 50000, 512
B, C = 8192, 32
N_CORES = 8
B_CORE = B // N_CORES          # 1024
N_GROUPS = B_CORE // 128       # 8
P = 128

f32 = mybir.dt.float32
i32 = mybir.dt.int32

_NC_CACHE = None


def _build():
    nc = bacc.Bacc(
        "TRN2", target_bir_lowering=False, debug=False, num_devices=N_CORES
    )
    w_d = nc.dram_tensor("w", [V, D], f32, kind="ExternalInput")
    ctx_idx_d = nc.dram_tensor("ctx_idx", [P, N_GROUPS * C], i32, kind="ExternalInput")
    cen_idx_d = nc.dram_tensor("cen_idx", [P, N_GROUPS], i32, kind="ExternalInput")
    out_d = nc.dram_tensor("out", [B_CORE, D], f32, kind="ExternalOutput")

    AF = mybir.ActivationFunctionType
    OP = mybir.AluOpType

    with tile.TileContext(nc) as tc, ExitStack() as ctx:
        const = ctx.enter_context(tc.tile_pool(name="const", bufs=1))
        big = ctx.enter_context(tc.tile_pool(name="big", bufs=2))
        med = ctx.enter_context(tc.tile_pool(name="med", bufs=2))
        stats = ctx.enter_context(tc.tile_pool(name="stats", bufs=2))

        idx_t = const.tile([P, N_GROUPS * C], i32)
        nc.sync.dma_start(out=idx_t[:], in_=ctx_idx_d[:])
        cidx_t = const.tile([P, N_GROUPS], i32)
        nc.sync.dma_start(out=cidx_t[:], in_=cen_idx_d[:])

        for g in range(N_GROUPS):
            # gather all 32 context rows per batch (partition = batch)
            ctx_all = big.tile([P, C * D], f32, tag="ctx")
            for c in range(C):
                nc.gpsimd.indirect_dma_start(
                    out=ctx_all[:, c * D : (c + 1) * D],
                    out_offset=None,
                    in_=w_d[:],
                    in_offset=bass.IndirectOffsetOnAxis(
                        ap=idx_t[:, g * C + c : g * C + c + 1], axis=0
                    ),
                )
            cen = med.tile([P, D], f32, tag="cen")
            nc.gpsimd.indirect_dma_start(
                out=cen[:],
                out_offset=None,
                in_=w_d[:],
                in_offset=bass.IndirectOffsetOnAxis(ap=cidx_t[:, g : g + 1], axis=0),
            )

            # squared distances -> dsq [128, 32]
            dsq = stats.tile([P, C], f32, tag="dsq")
            for c in range(C):
                sl = ctx_all[:, c * D : (c + 1) * D]
                diff = med.tile([P, D], f32, tag="diff")
                nc.vector.tensor_tensor(
                    out=diff[:], in0=sl, in1=cen[:], op=OP.subtract
                )
                sq = med.tile([P, D], f32, tag="sq")
                nc.scalar.activation(
                    out=sq[:], in_=diff[:], func=AF.Square,
                    accum_out=dsq[:, c : c + 1],
                )

            # weights
            w_t = stats.tile([P, C], f32, tag="w")
            nc.scalar.activation(out=w_t[:], in_=dsq[:], func=AF.Exp, scale=-0.5)

            den = stats.tile([P, 1], f32, tag="den")
            nc.vector.tensor_reduce(
                out=den[:], in_=w_t[:], axis=mybir.AxisListType.X, op=OP.add
            )
            den2 = stats.tile([P, 1], f32, tag="den2")
            nc.vector.tensor_scalar_add(den2[:], den[:], 1e-8)
            rcp = stats.tile([P, 1], f32, tag="rcp")
            nc.vector.reciprocal(out=rcp[:], in_=den2[:])

            # weighted sum of context rows
            acc = med.tile([P, D], f32, tag="acc")
            nc.scalar.mul(acc[:], ctx_all[:, 0:D], w_t[:, 0:1])
            for c in range(1, C):
                prod = med.tile([P, D], f32, tag="prod")
                nc.scalar.mul(
                    prod[:], ctx_all[:, c * D : (c + 1) * D], w_t[:, c : c + 1]
                )
                nc.vector.tensor_tensor(
                    out=acc[:], in0=acc[:], in1=prod[:], op=OP.add
                )

            out_sb = med.tile([P, D], f32, tag="osb")
            nc.scalar.mul(out_sb[:], acc[:], rcp[:, 0:1])
            nc.sync.dma_start(out=out_d[g * P : (g + 1) * P, :], in_=out_sb[:])

    nc.compile()
    return nc


def kernel(context, center, W):
    global _NC_CACHE
    from concourse.bass_utils import run_bass_kernel_spmd

    context = np.asarray(context)
    center = np.asarray(center)
    W = np.ascontiguousarray(np.asarray(W, dtype=np.float32))

    if _NC_CACHE is None:
        _NC_CACHE = _build()
    nc = _NC_CACHE

    in_maps = []
    for core in range(N_CORES):
        base = core * B_CORE
        ctx_blk = context[base : base + B_CORE].astype(np.int32)  # [1024, 32]
        cen_blk = center[base : base + B_CORE].astype(np.int32)   # [1024]
        # [p, g*C + c] = context[base + g*128 + p, c]
        ctx_idx = np.ascontiguousarray(
            ctx_blk.reshape(N_GROUPS, P, C).transpose(1, 0, 2).reshape(P, N_GROUPS * C)
        )
        # [p, g] = center[base + g*128 + p]
        cen_idx = np.ascontiguousarray(cen_blk.reshape(N_GROUPS, P).T)
        in_maps.append({"w": W, "ctx_idx": ctx_idx, "cen_idx": cen_idx})

    res = run_bass_kernel_spmd(nc, in_maps, list(range(N_CORES)))
    out = np.concatenate(
        [res.results[core]["out"] for core in range(N_CORES)], axis=0
    )
    return out.astype(np.float32)

